# revision 21
# baseline (speedup 1.0000x reference)
"""A3C loss kernel for Trainium2 (8 NeuronCores, data-parallel over batch).

The reference is a reverse scan over T=128 timesteps per trajectory:
    R_t   = sum_{s>=t} g^(s-t) r_s + g^(T-t) R0
    gae_t = R_t - v_t  (lambda=1 GAE telescopes to the advantage)
    critic = 0.5 * sum_t (R_t - v_t)^2
    actor  = -sum_t lp_t * gae_t - beta * sum_{t,a} ent
The suffix scan is a matmul with a [T,T] discount matrix, so the whole loss
is DMA + reductions + one small matmul per 128-row block.

The kernel is HBM-stream-bound (~75.5 MB of input per core; measured DMA
ceiling ~420 GB/s when all three DMA queues stay fed -> ~180 us stream floor).
Everything is organized around that:
  - three input streams, one DMA queue each: log_probs on the sync HWDGE
    ring, entropies on the SWDGE (gpsimd) queue with f32->bf16 cast during
    DMA, values/rewards (+ the tiny setup tensors) on the scalar HWDGE ring.
  - deep buffering (5 groups on each big stream) so no queue's issue stream
    is ever gated by a consumer hiccup - queue starvation shows up as lost
    aggregate bandwidth, which costs far more than SBUF.
  - per-group consumer cost must stay under the ~11 us group arrival
    cadence on every engine: vector carries the reductions (the bf16 cast
    halves the entropy-reduce cost), scalar only rt-copy + square, gpsimd
    only emits SWDGE descriptors, tensor does transposes + matmuls.
  - the final group is loaded per-block so its compute chases the last
    arriving bytes instead of adding a full group latency to the tail.

Layout: each core owns BC=8192 rows; block k (0..63), partition p holds local
row p*64+k, which makes every grouped load contiguous per partition and the
final [BC,2] output a single contiguous DMA from a [128, 128] staging tile.
"""

import numpy as np
from contextlib import ExitStack

import concourse.bacc as bacc
import concourse.bass as bass
import concourse.tile as tile
from concourse import mybir
from concourse.bass_utils import run_bass_kernel_spmd

GAMMA = 0.99
BETA = 0.01
B, T, A = 65536, 128, 8
N_CORES = 8
BC = B // N_CORES

G = 4  # row-blocks per streamed load group

F32 = mybir.dt.float32
BF16 = mybir.dt.bfloat16
ALU = mybir.AluOpType
ACTF = mybir.ActivationFunctionType
AXIS_X = mybir.AxisListType.X


def _discount_matrix() -> np.ndarray:
    # L[s, t] = gamma^(s-t) for s >= t else 0
    s = np.arange(T, dtype=np.float64)[:, None]
    t = np.arange(T, dtype=np.float64)[None, :]
    m = np.where(s >= t, GAMMA ** np.maximum(s - t, 0.0), 0.0)
    return m.astype(np.float32)


def build_nc(bc: int = BC):
    kb = bc // 128          # row-blocks
    ng = kb // G            # streamed groups
    nch = max(1, kb // 16)  # v/r chunks (16 blocks = 1 MiB of rewards each)
    jb = kb // nch          # blocks per chunk
    gpc = ng // nch         # groups per chunk
    assert bc % 128 == 0 and kb % G == 0 and ng % nch == 0 and jb % G == 0

    nc = bacc.Bacc("TRN2", target_bir_lowering=False, debug=False)

    v_d = nc.dram_tensor("values", [bc, T], F32, kind="ExternalInput")
    lv_d = nc.dram_tensor("last_value", [bc], F32, kind="ExternalInput")
    r_d = nc.dram_tensor("rewards", [bc, T], F32, kind="ExternalInput")
    lp_d = nc.dram_tensor("log_probs", [bc, T, A], F32, kind="ExternalInput")
    en_d = nc.dram_tensor("entropies", [bc, T, A], F32, kind="ExternalInput")
    tm_d = nc.dram_tensor("terminal_mask", [bc], mybir.dt.uint8, kind="ExternalInput")
    out_d = nc.dram_tensor("out", [bc, 2], F32, kind="ExternalOutput")

    lgam_d = nc.inline_tensor(_discount_matrix(), "lgam")
    iden_d = nc.inline_tensor(np.eye(128, dtype=np.float32), "iden")

    # local row index = p*kb + g*G + b -> grouped loads are contiguous per
    # partition (G and jb consecutive rows respectively)
    lp_view = lp_d.rearrange("(p g b) t a -> g p (b t) a", g=ng, b=G)
    en_view = en_d.rearrange("(p g b) t a -> g p b (t a)", g=ng, b=G)
    v_view = v_d.rearrange("(p c j) t -> c p (j t)", c=nch, j=jb)
    r_view = r_d.rearrange("(p c j) t -> c p j t", c=nch, j=jb)
    lv_view = lv_d.rearrange("(p k) -> p k", k=kb)
    tm_view = tm_d.rearrange("(p k) -> p k", k=kb)
    out_view = out_d.rearrange("(p k) j -> p (k j)", k=kb)

    with tile.TileContext(nc) as tc, ExitStack() as ctx:
        singles = ctx.enter_context(tc.tile_pool(name="singles", bufs=1))
        lp_pool = ctx.enter_context(tc.tile_pool(name="lp", bufs=5))
        en_pool = ctx.enter_context(tc.tile_pool(name="en", bufs=5))
        vr_pool = ctx.enter_context(tc.tile_pool(name="vr", bufs=2))
        work = ctx.enter_context(tc.tile_pool(name="work", bufs=2))
        small = ctx.enter_context(tc.tile_pool(name="small", bufs=4))
        psum_t = ctx.enter_context(tc.tile_pool(name="psum_t", bufs=2, space="PSUM"))
        psum_r = ctx.enter_context(tc.tile_pool(name="psum_r", bufs=2, space="PSUM"))

        # setup tensors ride the scalar HWDGE ring (it only carries the small
        # v/r streams), keeping sync pure-lp and the SWDGE queue pure-ent
        lgam_s = singles.tile([128, 128], F32)
        nc.scalar.dma_start(out=lgam_s, in_=lgam_d[:])
        iden_s = singles.tile([128, 128], F32)
        nc.scalar.dma_start(out=iden_s, in_=iden_d[:])
        lv_s = singles.tile([128, kb], F32)
        nc.scalar.dma_start(out=lv_s, in_=lv_view)
        tm_s = singles.tile([128, kb], mybir.dt.uint8)
        nc.scalar.dma_start(out=tm_s, in_=tm_view)

        # gr0 = gamma * last_value * (1 - mask), on vector (idle at start;
        # gpsimd must stay a pure SWDGE-emission engine)
        tmf = singles.tile([128, kb], F32)
        nc.vector.tensor_copy(out=tmf, in_=tm_s)
        lvm = singles.tile([128, kb], F32)
        nc.vector.tensor_mul(lvm, lv_s, tmf)
        gr0 = singles.tile([128, kb], F32)
        nc.vector.tensor_sub(gr0, lv_s, lvm)
        nc.vector.tensor_scalar_mul(gr0, gr0, GAMMA)

        stage = singles.tile([128, 2 * kb], F32)
        stage_kj = stage.rearrange("p (k j) -> p k j", j=2)

        v_t: list = []
        r_t: list = []

        for g in range(ng):
            if g % gpc == 0:
                # prefetch the next v/r chunk on the scalar ring; fold
                # gamma*R0 into the last reward column once per chunk
                c = g // gpc
                rt = vr_pool.tile([128, jb, T], F32)
                nc.scalar.dma_start(out=rt, in_=r_view[c])
                vt = vr_pool.tile([128, jb * T], F32)
                nc.scalar.dma_start(out=vt, in_=v_view[c])
                nc.vector.tensor_tensor(
                    out=rt[:, :, T - 1],
                    in0=rt[:, :, T - 1],
                    in1=gr0[:, c * jb : (c + 1) * jb],
                    op=ALU.add,
                )
                r_t.append(rt)
                v_t.append(vt)

            c = g // gpc
            j0 = (g % gpc) * G
            k0 = g * G

            last = g == ng - 1

            lp_t = lp_pool.tile([128, G * T, A], F32)
            # entropies are cast f32->bf16 during the SWDGE DMA: HBM reads
            # are unchanged but the vector reduce over them runs at 2x
            en_t = en_pool.tile([128, G, T * A], BF16)
            if last:
                # the final group arrives as per-block sub-DMAs so its
                # compute pipelines against the arriving blocks instead of
                # waiting out the full transfer after the stream ends
                for b in range(G):
                    nc.sync.dma_start(
                        out=lp_t[:, b * T : (b + 1) * T, :],
                        in_=lp_view[g][:, b * T : (b + 1) * T, :],
                    )
                    nc.gpsimd.dma_start(out=en_t[:, b, :], in_=en_view[g][:, b, :])
            else:
                nc.sync.dma_start(out=lp_t, in_=lp_view[g])
                nc.gpsimd.dma_start(out=en_t, in_=en_view[g])

            # time-major rewards for the scan matmuls; PSUM->SBUF copy on
            # scalar (its only other op is the square - both ping-pong with
            # tensor/vector but scalar has huge slack)
            rt_ps = psum_t.tile([128, G * T], F32)
            for b in range(G):
                nc.tensor.transpose(
                    rt_ps[:, b * T : (b + 1) * T], r_t[c][:, j0 + b, :], iden_s
                )
            rt_sb = work.tile([128, G * T], F32)
            nc.scalar.activation(out=rt_sb, in_=rt_ps, func=ACTF.Copy, bias=0.0, scale=1.0)

            # lp[b, t] = sum_a log_probs (for the last group this moves after
            # the reward chain, per-block - see below)
            lps = work.tile([128, G * T], F32)
            if not last:
                nc.vector.reduce_sum(out=lps, in_=lp_t, axis=AXIS_X)

            # ents[b] = sum_{t,a} entropies (raw; -beta folds into the final
            # combine), on vector - bf16 input halves the cost
            ents = small.tile([128, G], F32)
            if not last:
                nc.vector.reduce_sum(out=ents, in_=en_t, axis=AXIS_X)

            # R[b, t] = sum_s r'T[s, b] * Lgam[s, t]
            R_ps = psum_r.tile([128, G * T], F32)
            for b in range(G):
                nc.tensor.matmul(
                    R_ps[:, b * T : (b + 1) * T],
                    lhsT=rt_sb[:, b * T : (b + 1) * T],
                    rhs=lgam_s,
                    start=True,
                    stop=True,
                )

            adv = work.tile([128, G * T], F32)
            nc.vector.tensor_sub(adv, R_ps, v_t[c][:, j0 * T : (j0 + G) * T])

            # critic = 0.5 * sum_t adv^2: square on scalar, reduce on vector.
            # The whole reward chain has no lp/en dependency, so for the last
            # group it completes before the stream ends.
            sq = work.tile([128, G * T], F32)
            nc.scalar.activation(
                out=sq, in_=adv, func=ACTF.Square, bias=0.0,
                scale=float(np.sqrt(0.5)),
            )
            nc.vector.reduce_sum(
                out=stage_kj[:, k0 : k0 + G, 1],
                in_=sq.rearrange("p (b t) -> p b t", t=T),
                axis=AXIS_X,
            )

            # actor = -sum_t lp*adv - beta*sum ent
            prod = work.tile([128, G * T], F32)
            acc = small.tile([128, G], F32)
            if last:
                # per-block, fused with the accumulation, chasing the
                # arriving sub-DMAs
                for b in range(G):
                    nc.vector.reduce_sum(
                        out=lps[:, b * T : (b + 1) * T],
                        in_=lp_t[:, b * T : (b + 1) * T, :],
                        axis=AXIS_X,
                    )
                    nc.vector.scalar_tensor_tensor(
                        out=prod[:, b * T : (b + 1) * T],
                        in0=adv[:, b * T : (b + 1) * T], scalar=-1.0,
                        in1=lps[:, b * T : (b + 1) * T],
                        op0=ALU.mult, op1=ALU.mult,
                        accum_out=acc[:, b : b + 1],
                    )
                    nc.vector.reduce_sum(
                        out=ents[:, b : b + 1], in_=en_t[:, b : b + 1, :],
                        axis=AXIS_X,
                    )
            else:
                nc.vector.scalar_tensor_tensor(
                    out=prod, in0=adv, scalar=-1.0, in1=lps,
                    op0=ALU.mult, op1=ALU.mult,
                )
                nc.vector.reduce_sum(
                    out=acc, in_=prod.rearrange("p (b t) -> p b t", t=T), axis=AXIS_X
                )
            nc.vector.scalar_tensor_tensor(
                out=stage_kj[:, k0 : k0 + G, 0], in0=ents, scalar=-BETA, in1=acc,
                op0=ALU.mult, op1=ALU.add,
            )

        # single store at the end on the sync ring (drained of loads by then)
        nc.sync.dma_start(out=out_view, in_=stage)

    nc.compile()
    return nc


_NC = None


def _get_nc():
    global _NC
    if _NC is None:
        _NC = build_nc(BC)
    return _NC


def _make_in_maps(inputs: dict) -> list[dict]:
    v = np.ascontiguousarray(np.asarray(inputs["values"], dtype=np.float32))
    lv = np.ascontiguousarray(np.asarray(inputs["last_value"], dtype=np.float32))
    r = np.ascontiguousarray(np.asarray(inputs["rewards"], dtype=np.float32))
    lp = np.ascontiguousarray(np.asarray(inputs["log_probs"], dtype=np.float32))
    en = np.ascontiguousarray(np.asarray(inputs["entropies"], dtype=np.float32))
    tm = np.ascontiguousarray(np.asarray(inputs["terminal_mask"]).astype(np.uint8))
    maps = []
    for c in range(N_CORES):
        sl = slice(c * BC, (c + 1) * BC)
        maps.append(
            {
                "values": v[sl],
                "last_value": lv[sl],
                "rewards": r[sl],
                "log_probs": lp[sl],
                "entropies": en[sl],
                "terminal_mask": tm[sl],
            }
        )
    return maps


def _run(inputs: dict, trace: bool = False):
    nc = _get_nc()
    res = run_bass_kernel_spmd(
        nc,
        _make_in_maps(inputs),
        core_ids=list(range(N_CORES)),
        trace=trace,
    )
    out = np.concatenate([res.results[c]["out"] for c in range(N_CORES)], axis=0)
    return out, res


def kernel(**inputs) -> np.ndarray:
    out, _ = _run(inputs, trace=False)
    return out


# revision 22
# speedup vs baseline: 1.1118x; 1.1118x over previous
"""A3C loss kernel for Trainium2 (8 NeuronCores, data-parallel over batch).

The reference is a reverse scan over T=128 timesteps per trajectory:
    R_t   = sum_{s>=t} g^(s-t) r_s + g^(T-t) R0
    gae_t = R_t - v_t  (lambda=1 GAE telescopes to the advantage)
    critic = 0.5 * sum_t (R_t - v_t)^2
    actor  = -sum_t lp_t * gae_t - beta * sum_{t,a} ent
The suffix scan is a matmul with a [T,T] discount matrix, so the whole loss
is DMA + reductions + one small matmul per 128-row block.

The kernel is HBM-stream-bound (~75.5 MB of input per core; measured DMA
ceiling ~420 GB/s when both HWDGE rings stay fed -> ~180 us stream floor).
Organization:
  - log_probs (+rewards) stream on the sync HWDGE ring, entropies (+values)
    on the scalar HWDGE ring, in groups of 4 row-blocks (2 MiB per DMA,
    16 KiB contiguous per partition). SWDGE (gpsimd) carries only the tiny
    setup tensors. (SWDGE f32->bf16 cast-during-DMA was tried for the ent
    stream and cannot sustain line rate - don't go back there.)
  - deep buffering (5 groups on ent, 4 on lp) so neither ring's issue
    stream is gated by a consumer hiccup - ring starvation loses aggregate
    bandwidth (341 GB/s vs 420 measured), far more costly than SBUF.
  - per-group consumer cost stays under the ~11 us group arrival cadence on
    every engine, and the entropy sums are split 3:1 between scalar (act
    with accum) and vector (reduce) so the end-of-stream backlog drains in
    parallel. Scalar carries nothing that waits on tensor or vector.
  - the final group is loaded per-block so its compute chases the last
    arriving bytes instead of adding a full group latency to the tail.

Layout: each core owns BC=8192 rows; block k (0..63), partition p holds local
row p*64+k, which makes every grouped load contiguous per partition and the
final [BC,2] output a single contiguous DMA from a [128, 128] staging tile.
"""

import numpy as np
from contextlib import ExitStack

import concourse.bacc as bacc
import concourse.bass as bass
import concourse.tile as tile
from concourse import mybir
from concourse.bass_utils import run_bass_kernel_spmd

GAMMA = 0.99
BETA = 0.01
B, T, A = 65536, 128, 8
N_CORES = 8
BC = B // N_CORES

G = 4       # row-blocks per streamed load group
ENT_V = 1   # ent blocks per group reduced on vector (rest on scalar)

F32 = mybir.dt.float32
ALU = mybir.AluOpType
ACTF = mybir.ActivationFunctionType
AXIS_X = mybir.AxisListType.X


def _discount_matrix() -> np.ndarray:
    # L[s, t] = gamma^(s-t) for s >= t else 0
    s = np.arange(T, dtype=np.float64)[:, None]
    t = np.arange(T, dtype=np.float64)[None, :]
    m = np.where(s >= t, GAMMA ** np.maximum(s - t, 0.0), 0.0)
    return m.astype(np.float32)


def build_nc(bc: int = BC):
    kb = bc // 128          # row-blocks
    ng = kb // G            # streamed groups
    nch = max(1, kb // 16)  # v/r chunks (16 blocks = 1 MiB of rewards each)
    jb = kb // nch          # blocks per chunk
    gpc = ng // nch         # groups per chunk
    assert bc % 128 == 0 and kb % G == 0 and ng % nch == 0 and jb % G == 0

    nc = bacc.Bacc("TRN2", target_bir_lowering=False, debug=False)

    v_d = nc.dram_tensor("values", [bc, T], F32, kind="ExternalInput")
    lv_d = nc.dram_tensor("last_value", [bc], F32, kind="ExternalInput")
    r_d = nc.dram_tensor("rewards", [bc, T], F32, kind="ExternalInput")
    lp_d = nc.dram_tensor("log_probs", [bc, T, A], F32, kind="ExternalInput")
    en_d = nc.dram_tensor("entropies", [bc, T, A], F32, kind="ExternalInput")
    tm_d = nc.dram_tensor("terminal_mask", [bc], mybir.dt.uint8, kind="ExternalInput")
    out_d = nc.dram_tensor("out", [bc, 2], F32, kind="ExternalOutput")

    lgam_d = nc.inline_tensor(_discount_matrix(), "lgam")
    iden_d = nc.inline_tensor(np.eye(128, dtype=np.float32), "iden")

    # local row index = p*kb + g*G + b -> grouped loads are contiguous per
    # partition (G and jb consecutive rows respectively)
    lp_view = lp_d.rearrange("(p g b) t a -> g p (b t) a", g=ng, b=G)
    en_view = en_d.rearrange("(p g b) t a -> g p b (t a)", g=ng, b=G)
    v_view = v_d.rearrange("(p c j) t -> c p (j t)", c=nch, j=jb)
    r_view = r_d.rearrange("(p c j) t -> c p j t", c=nch, j=jb)
    lv_view = lv_d.rearrange("(p k) -> p k", k=kb)
    tm_view = tm_d.rearrange("(p k) -> p k", k=kb)
    out_view = out_d.rearrange("(p k) j -> p (k j)", k=kb)

    with tile.TileContext(nc) as tc, ExitStack() as ctx:
        singles = ctx.enter_context(tc.tile_pool(name="singles", bufs=1))
        lp_pool = ctx.enter_context(tc.tile_pool(name="lp", bufs=4))
        en_pool = ctx.enter_context(tc.tile_pool(name="en", bufs=5))
        vr_pool = ctx.enter_context(tc.tile_pool(name="vr", bufs=2))
        work = ctx.enter_context(tc.tile_pool(name="work", bufs=2))
        scr = ctx.enter_context(tc.tile_pool(name="scr", bufs=1))
        small = ctx.enter_context(tc.tile_pool(name="small", bufs=4))
        psum_t = ctx.enter_context(tc.tile_pool(name="psum_t", bufs=2, space="PSUM"))
        psum_r = ctx.enter_context(tc.tile_pool(name="psum_r", bufs=2, space="PSUM"))

        # singles go through SWDGE (gpsimd) so the two HWDGE rings are free
        # for the streamed loads from instruction zero
        lgam_s = singles.tile([128, 128], F32)
        nc.gpsimd.dma_start(out=lgam_s, in_=lgam_d[:])
        iden_s = singles.tile([128, 128], F32)
        nc.gpsimd.dma_start(out=iden_s, in_=iden_d[:])
        lv_s = singles.tile([128, kb], F32)
        nc.gpsimd.dma_start(out=lv_s, in_=lv_view)
        tm_s = singles.tile([128, kb], mybir.dt.uint8)
        nc.gpsimd.dma_start(out=tm_s, in_=tm_view)

        # gr0 = gamma * last_value * (1 - mask)
        tmf = singles.tile([128, kb], F32)
        nc.gpsimd.tensor_copy(out=tmf, in_=tm_s)
        lvm = singles.tile([128, kb], F32)
        nc.gpsimd.tensor_mul(lvm, lv_s, tmf)
        gr0 = singles.tile([128, kb], F32)
        nc.gpsimd.tensor_sub(gr0, lv_s, lvm)
        nc.gpsimd.tensor_scalar_mul(gr0, gr0, GAMMA)

        stage = singles.tile([128, 2 * kb], F32)
        stage_kj = stage.rearrange("p (k j) -> p k j", j=2)

        v_t: list = []
        r_t: list = []

        for g in range(ng):
            if g % gpc == 0:
                # prefetch the next v/r chunk into each ring ahead of the
                # groups that consume it; fold gamma*R0 into the last reward
                # column once per chunk (one strided op instead of per-block)
                c = g // gpc
                rt = vr_pool.tile([128, jb, T], F32)
                nc.sync.dma_start(out=rt, in_=r_view[c])
                vt = vr_pool.tile([128, jb * T], F32)
                nc.scalar.dma_start(out=vt, in_=v_view[c])
                nc.gpsimd.tensor_tensor(
                    out=rt[:, :, T - 1],
                    in0=rt[:, :, T - 1],
                    in1=gr0[:, c * jb : (c + 1) * jb],
                    op=ALU.add,
                )
                r_t.append(rt)
                v_t.append(vt)

            c = g // gpc
            j0 = (g % gpc) * G
            k0 = g * G

            last = g == ng - 1

            lp_t = lp_pool.tile([128, G * T, A], F32)
            en_t = en_pool.tile([128, G, T * A], F32)
            if last:
                # the final group arrives as per-block sub-DMAs so its
                # compute pipelines against the arriving blocks instead of
                # waiting out the full 2 MiB transfer after the stream ends
                for b in range(G):
                    nc.sync.dma_start(
                        out=lp_t[:, b * T : (b + 1) * T, :],
                        in_=lp_view[g][:, b * T : (b + 1) * T, :],
                    )
                    nc.scalar.dma_start(out=en_t[:, b, :], in_=en_view[g][:, b, :])
            else:
                nc.sync.dma_start(out=lp_t, in_=lp_view[g])
                nc.scalar.dma_start(out=en_t, in_=en_view[g])

            # time-major rewards for the scan matmuls. The PSUM->SBUF copy
            # runs on vector as the group's FIRST vector op: scalar must
            # carry nothing that waits on tensor/vector — its stream is
            # purely [dma][ent acts] so the ent ring never lags.
            rt_ps = psum_t.tile([128, G * T], F32)
            for b in range(G):
                nc.tensor.transpose(
                    rt_ps[:, b * T : (b + 1) * T], r_t[c][:, j0 + b, :], iden_s
                )
            rt_sb = work.tile([128, G * T], F32)
            nc.vector.tensor_copy(out=rt_sb, in_=rt_ps)

            # lp[b, t] = sum_a log_probs (for the last group this moves after
            # the reward chain, per-block — see below)
            lps = work.tile([128, G * T], F32)
            if not last:
                nc.vector.reduce_sum(out=lps, in_=lp_t, axis=AXIS_X)

            # ents[b] = sum_{t,a} entropies (raw; -beta folds into the final
            # combine), split 3:1 scalar:vector so the end-of-stream ent
            # backlog drains on two engines in parallel. The scalar scratch
            # out is a write-only dummy (scalar is in-order, WAW is free).
            ents = small.tile([128, G], F32)
            entscr = scr.tile([128, T * A], F32)
            for b in range(G - ENT_V):
                nc.scalar.activation(
                    out=entscr,
                    in_=en_t[:, b, :],
                    func=ACTF.Copy, bias=0.0, scale=1.0,
                    accum_out=ents[:, b : b + 1],
                )
            for b in range(G - ENT_V, G):
                nc.vector.reduce_sum(
                    out=ents[:, b : b + 1], in_=en_t[:, b : b + 1, :], axis=AXIS_X
                )

            # R[b, t] = sum_s r'T[s, b] * Lgam[s, t]
            R_ps = psum_r.tile([128, G * T], F32)
            for b in range(G):
                nc.tensor.matmul(
                    R_ps[:, b * T : (b + 1) * T],
                    lhsT=rt_sb[:, b * T : (b + 1) * T],
                    rhs=lgam_s,
                    start=True,
                    stop=True,
                )

            adv = work.tile([128, G * T], F32)
            nc.vector.tensor_sub(adv, R_ps, v_t[c][:, j0 * T : (j0 + G) * T])

            # critic = 0.5 * sum_t adv^2 (square on vector, not scalar — the
            # square waits on adv and must not sit in scalar's stream). The
            # whole reward chain has no lp/en dependency, so for the last
            # group it completes before the stream ends.
            sq = work.tile([128, G * T], F32)
            nc.vector.scalar_tensor_tensor(
                out=sq, in0=adv, scalar=0.5, in1=adv,
                op0=ALU.mult, op1=ALU.mult,
            )
            nc.vector.reduce_sum(
                out=stage_kj[:, k0 : k0 + G, 1],
                in_=sq.rearrange("p (b t) -> p b t", t=T),
                axis=AXIS_X,
            )

            # actor = -sum_t lp*adv - beta*sum ent
            prod = work.tile([128, G * T], F32)
            acc = small.tile([128, G], F32)
            if last:
                # per-block, fused with the accumulation, chasing the
                # arriving sub-DMAs
                for b in range(G):
                    nc.vector.reduce_sum(
                        out=lps[:, b * T : (b + 1) * T],
                        in_=lp_t[:, b * T : (b + 1) * T, :],
                        axis=AXIS_X,
                    )
                    nc.vector.scalar_tensor_tensor(
                        out=prod[:, b * T : (b + 1) * T],
                        in0=adv[:, b * T : (b + 1) * T], scalar=-1.0,
                        in1=lps[:, b * T : (b + 1) * T],
                        op0=ALU.mult, op1=ALU.mult,
                        accum_out=acc[:, b : b + 1],
                    )
            else:
                nc.vector.scalar_tensor_tensor(
                    out=prod, in0=adv, scalar=-1.0, in1=lps,
                    op0=ALU.mult, op1=ALU.mult,
                )
                nc.vector.reduce_sum(
                    out=acc, in_=prod.rearrange("p (b t) -> p b t", t=T), axis=AXIS_X
                )
            nc.vector.scalar_tensor_tensor(
                out=stage_kj[:, k0 : k0 + G, 0], in0=ents, scalar=-BETA, in1=acc,
                op0=ALU.mult, op1=ALU.add,
            )

        # single store at the end on the sync ring (drained of loads by then)
        nc.sync.dma_start(out=out_view, in_=stage)

    nc.compile()
    return nc


_NC = None


def _get_nc():
    global _NC
    if _NC is None:
        _NC = build_nc(BC)
    return _NC


def _make_in_maps(inputs: dict) -> list[dict]:
    v = np.ascontiguousarray(np.asarray(inputs["values"], dtype=np.float32))
    lv = np.ascontiguousarray(np.asarray(inputs["last_value"], dtype=np.float32))
    r = np.ascontiguousarray(np.asarray(inputs["rewards"], dtype=np.float32))
    lp = np.ascontiguousarray(np.asarray(inputs["log_probs"], dtype=np.float32))
    en = np.ascontiguousarray(np.asarray(inputs["entropies"], dtype=np.float32))
    tm = np.ascontiguousarray(np.asarray(inputs["terminal_mask"]).astype(np.uint8))
    maps = []
    for c in range(N_CORES):
        sl = slice(c * BC, (c + 1) * BC)
        maps.append(
            {
                "values": v[sl],
                "last_value": lv[sl],
                "rewards": r[sl],
                "log_probs": lp[sl],
                "entropies": en[sl],
                "terminal_mask": tm[sl],
            }
        )
    return maps


def _run(inputs: dict, trace: bool = False):
    nc = _get_nc()
    res = run_bass_kernel_spmd(
        nc,
        _make_in_maps(inputs),
        core_ids=list(range(N_CORES)),
        trace=trace,
    )
    out = np.concatenate([res.results[c]["out"] for c in range(N_CORES)], axis=0)
    return out, res


def kernel(**inputs) -> np.ndarray:
    out, _ = _run(inputs, trace=False)
    return out


# revision 25
# speedup vs baseline: 1.3269x; 1.1935x over previous
"""A3C loss kernel for Trainium2 (8 NeuronCores, data-parallel over batch).

The reference is a reverse scan over T=128 timesteps per trajectory:
    R_t   = sum_{s>=t} g^(s-t) r_s + g^(T-t) R0
    gae_t = R_t - v_t  (lambda=1 GAE telescopes to the advantage)
    critic = 0.5 * sum_t (R_t - v_t)^2
    actor  = -sum_t lp_t * gae_t - beta * sum_{t,a} ent
The suffix scan is a matmul with a [T,T] discount matrix, so the whole loss
is DMA + reductions + one small matmul per 128-row block.

The kernel is HBM-stream-bound (~75.5 MB of input per core; measured DMA
ceiling ~420 GB/s when both HWDGE rings stay fed -> ~180 us stream floor).
Organization:
  - log_probs (+rewards) stream on the sync HWDGE ring, entropies (+values)
    on the scalar HWDGE ring, in groups of 4 row-blocks (2 MiB per DMA,
    16 KiB contiguous per partition). SWDGE (gpsimd) carries only the tiny
    setup tensors. (SWDGE f32->bf16 cast-during-DMA was tried for the ent
    stream and cannot sustain line rate - don't go back there.)
  - deep buffering (5 groups on ent, 4 on lp) so neither ring's issue
    stream is gated by a consumer hiccup - ring starvation loses aggregate
    bandwidth (341 GB/s vs 420 measured), far more costly than SBUF.
  - per-group consumer cost stays under the ~11 us group arrival cadence on
    every engine, and the entropy sums are split 3:1 between scalar (act
    with accum) and vector (reduce) so the end-of-stream backlog drains in
    parallel. Scalar carries nothing that waits on tensor or vector.
  - the final group is loaded per-block so its compute chases the last
    arriving bytes instead of adding a full group latency to the tail.

Layout: each core owns BC=8192 rows; block k (0..63), partition p holds local
row p*64+k, which makes every grouped load contiguous per partition and the
final [BC,2] output a single contiguous DMA from a [128, 128] staging tile.
"""

import numpy as np
from contextlib import ExitStack

import concourse.bacc as bacc
import concourse.bass as bass
import concourse.tile as tile
from concourse import mybir
from concourse.bass_utils import run_bass_kernel_spmd

GAMMA = 0.99
BETA = 0.01
B, T, A = 65536, 128, 8
N_CORES = 8
BC = B // N_CORES

G = 4       # row-blocks per streamed load group
ENT_V = 1   # ent blocks per group reduced on vector (rest on scalar)

F32 = mybir.dt.float32
ALU = mybir.AluOpType
ACTF = mybir.ActivationFunctionType
AXIS_X = mybir.AxisListType.X


def _discount_matrix() -> np.ndarray:
    # L[s, t] = gamma^(s-t) for s >= t else 0
    s = np.arange(T, dtype=np.float64)[:, None]
    t = np.arange(T, dtype=np.float64)[None, :]
    m = np.where(s >= t, GAMMA ** np.maximum(s - t, 0.0), 0.0)
    return m.astype(np.float32)


def build_nc(bc: int = BC):
    kb = bc // 128          # row-blocks
    ng = kb // G            # streamed groups
    nch = max(1, kb // 16)  # v/r chunks (16 blocks = 1 MiB of rewards each)
    jb = kb // nch          # blocks per chunk
    gpc = ng // nch         # groups per chunk
    assert bc % 128 == 0 and kb % G == 0 and ng % nch == 0 and jb % G == 0

    nc = bacc.Bacc("TRN2", target_bir_lowering=False, debug=False)

    v_d = nc.dram_tensor("values", [bc, T], F32, kind="ExternalInput")
    lv_d = nc.dram_tensor("last_value", [bc], F32, kind="ExternalInput")
    r_d = nc.dram_tensor("rewards", [bc, T], F32, kind="ExternalInput")
    lp_d = nc.dram_tensor("log_probs", [bc, T, A], F32, kind="ExternalInput")
    en_d = nc.dram_tensor("entropies", [bc, T, A], F32, kind="ExternalInput")
    tm_d = nc.dram_tensor("terminal_mask", [bc], mybir.dt.uint8, kind="ExternalInput")
    out_d = nc.dram_tensor("out", [bc, 2], F32, kind="ExternalOutput")

    lgam_d = nc.inline_tensor(_discount_matrix(), "lgam")
    iden_d = nc.inline_tensor(np.eye(128, dtype=np.float32), "iden")

    # local row index = p*kb + g*G + b -> grouped loads are contiguous per
    # partition (G and jb consecutive rows respectively)
    lp_view = lp_d.rearrange("(p g b) t a -> g p (b t) a", g=ng, b=G)
    en_view = en_d.rearrange("(p g b) t a -> g p b (t a)", g=ng, b=G)
    v_view = v_d.rearrange("(p c j) t -> c p (j t)", c=nch, j=jb)
    r_view = r_d.rearrange("(p c j) t -> c p j t", c=nch, j=jb)
    lv_view = lv_d.rearrange("(p k) -> p k", k=kb)
    tm_view = tm_d.rearrange("(p k) -> p k", k=kb)
    out_view = out_d.rearrange("(p k) j -> p (k j)", k=kb)

    with tile.TileContext(nc) as tc, ExitStack() as ctx:
        singles = ctx.enter_context(tc.tile_pool(name="singles", bufs=1))
        lp_pool = ctx.enter_context(tc.tile_pool(name="lp", bufs=3))
        # the ent group is split into two tiles: en_s consumed by scalar,
        # en_v by vector — each tile's ring slot is release-gated by exactly
        # ONE engine. A single shared tile read by both engines couples the
        # en ring to the vector loop and costs ~30 us of stream bandwidth.
        en_pool = ctx.enter_context(tc.tile_pool(name="en", bufs=5))
        env_pool = ctx.enter_context(tc.tile_pool(name="env", bufs=5))
        vr_pool = ctx.enter_context(tc.tile_pool(name="vr", bufs=2))
        work = ctx.enter_context(tc.tile_pool(name="work", bufs=2))
        scr = ctx.enter_context(tc.tile_pool(name="scr", bufs=1))
        small = ctx.enter_context(tc.tile_pool(name="small", bufs=4))
        psum_t = ctx.enter_context(tc.tile_pool(name="psum_t", bufs=2, space="PSUM"))
        psum_r = ctx.enter_context(tc.tile_pool(name="psum_r", bufs=2, space="PSUM"))

        # singles go through SWDGE (gpsimd) so the two HWDGE rings are free
        # for the streamed loads from instruction zero
        lgam_s = singles.tile([128, 128], F32)
        nc.gpsimd.dma_start(out=lgam_s, in_=lgam_d[:])
        iden_s = singles.tile([128, 128], F32)
        nc.gpsimd.dma_start(out=iden_s, in_=iden_d[:])
        lv_s = singles.tile([128, kb], F32)
        nc.gpsimd.dma_start(out=lv_s, in_=lv_view)
        tm_s = singles.tile([128, kb], mybir.dt.uint8)
        nc.gpsimd.dma_start(out=tm_s, in_=tm_view)

        # gr0 = gamma * last_value * (1 - mask)
        tmf = singles.tile([128, kb], F32)
        nc.gpsimd.tensor_copy(out=tmf, in_=tm_s)
        lvm = singles.tile([128, kb], F32)
        nc.gpsimd.tensor_mul(lvm, lv_s, tmf)
        gr0 = singles.tile([128, kb], F32)
        nc.gpsimd.tensor_sub(gr0, lv_s, lvm)
        nc.gpsimd.tensor_scalar_mul(gr0, gr0, GAMMA)

        stage = singles.tile([128, 2 * kb], F32)
        stage_kj = stage.rearrange("p (k j) -> p k j", j=2)

        v_t: list = []
        r_t: list = []

        for g in range(ng):
            if g % gpc == 0:
                # prefetch the next v/r chunk into each ring ahead of the
                # groups that consume it; fold gamma*R0 into the last reward
                # column once per chunk (one strided op instead of per-block)
                c = g // gpc
                rt = vr_pool.tile([128, jb, T], F32)
                nc.sync.dma_start(out=rt, in_=r_view[c])
                vt = vr_pool.tile([128, jb * T], F32)
                nc.scalar.dma_start(out=vt, in_=v_view[c])
                nc.gpsimd.tensor_tensor(
                    out=rt[:, :, T - 1],
                    in0=rt[:, :, T - 1],
                    in1=gr0[:, c * jb : (c + 1) * jb],
                    op=ALU.add,
                )
                r_t.append(rt)
                v_t.append(vt)

            c = g // gpc
            j0 = (g % gpc) * G
            k0 = g * G

            last = g == ng - 1

            GS = G - ENT_V  # ent blocks consumed by scalar
            lp_t = lp_pool.tile([128, G * T, A], F32)
            en_s = en_pool.tile([128, GS, T * A], F32)
            en_v = env_pool.tile([128, ENT_V, T * A], F32)
            if last:
                # the final group arrives as per-block sub-DMAs so its
                # compute pipelines against the arriving blocks instead of
                # waiting out the full 2 MiB transfer after the stream ends
                for b in range(G):
                    nc.sync.dma_start(
                        out=lp_t[:, b * T : (b + 1) * T, :],
                        in_=lp_view[g][:, b * T : (b + 1) * T, :],
                    )
                for b in range(GS):
                    nc.scalar.dma_start(out=en_s[:, b, :], in_=en_view[g][:, b, :])
                nc.scalar.dma_start(out=en_v, in_=en_view[g][:, GS:G, :])
            else:
                nc.sync.dma_start(out=lp_t, in_=lp_view[g])
                nc.scalar.dma_start(out=en_s, in_=en_view[g][:, 0:GS, :])
                nc.scalar.dma_start(out=en_v, in_=en_view[g][:, GS:G, :])

            # time-major rewards for the scan matmuls. The PSUM->SBUF copy
            # runs on vector as the group's FIRST vector op: scalar must
            # carry nothing that waits on tensor/vector — its stream is
            # purely [dma][ent acts] so the ent ring never lags.
            rt_ps = psum_t.tile([128, G * T], F32)
            for b in range(G):
                nc.tensor.transpose(
                    rt_ps[:, b * T : (b + 1) * T], r_t[c][:, j0 + b, :], iden_s
                )
            rt_sb = work.tile([128, G * T], F32)
            nc.vector.tensor_copy(out=rt_sb, in_=rt_ps)

            # lp[b, t] = sum_a log_probs (for the last group this moves after
            # the reward chain, per-block — see below)
            lps = work.tile([128, G * T], F32)
            if not last:
                nc.vector.reduce_sum(out=lps, in_=lp_t, axis=AXIS_X)

            # ents[b] = sum_{t,a} entropies (raw; -beta folds into the final
            # combine), split 3:1 scalar:vector so the end-of-stream ent
            # backlog drains on two engines in parallel. The scalar scratch
            # out is a write-only dummy (scalar is in-order, WAW is free).
            ents = small.tile([128, G], F32)
            entscr = scr.tile([128, T * A], F32)
            for b in range(GS):
                nc.scalar.activation(
                    out=entscr,
                    in_=en_s[:, b, :],
                    func=ACTF.Copy, bias=0.0, scale=1.0,
                    accum_out=ents[:, b : b + 1],
                )
            nc.vector.reduce_sum(out=ents[:, GS:G], in_=en_v, axis=AXIS_X)

            # R[b, t] = sum_s r'T[s, b] * Lgam[s, t]
            R_ps = psum_r.tile([128, G * T], F32)
            for b in range(G):
                nc.tensor.matmul(
                    R_ps[:, b * T : (b + 1) * T],
                    lhsT=rt_sb[:, b * T : (b + 1) * T],
                    rhs=lgam_s,
                    start=True,
                    stop=True,
                )

            adv = work.tile([128, G * T], F32)
            nc.vector.tensor_sub(adv, R_ps, v_t[c][:, j0 * T : (j0 + G) * T])

            # critic = 0.5 * sum_t adv^2 (square on vector, not scalar — the
            # square waits on adv and must not sit in scalar's stream). The
            # whole reward chain has no lp/en dependency, so for the last
            # group it completes before the stream ends.
            sq = work.tile([128, G * T], F32)
            nc.vector.scalar_tensor_tensor(
                out=sq, in0=adv, scalar=0.5, in1=adv,
                op0=ALU.mult, op1=ALU.mult,
            )
            nc.vector.reduce_sum(
                out=stage_kj[:, k0 : k0 + G, 1],
                in_=sq.rearrange("p (b t) -> p b t", t=T),
                axis=AXIS_X,
            )

            # actor = -sum_t lp*adv - beta*sum ent
            prod = work.tile([128, G * T], F32)
            acc = small.tile([128, G], F32)
            if last:
                # per-block, fused with the accumulation, chasing the
                # arriving sub-DMAs
                for b in range(G):
                    nc.vector.reduce_sum(
                        out=lps[:, b * T : (b + 1) * T],
                        in_=lp_t[:, b * T : (b + 1) * T, :],
                        axis=AXIS_X,
                    )
                    nc.vector.scalar_tensor_tensor(
                        out=prod[:, b * T : (b + 1) * T],
                        in0=adv[:, b * T : (b + 1) * T], scalar=-1.0,
                        in1=lps[:, b * T : (b + 1) * T],
                        op0=ALU.mult, op1=ALU.mult,
                        accum_out=acc[:, b : b + 1],
                    )
            else:
                nc.vector.scalar_tensor_tensor(
                    out=prod, in0=adv, scalar=-1.0, in1=lps,
                    op0=ALU.mult, op1=ALU.mult,
                )
                nc.vector.reduce_sum(
                    out=acc, in_=prod.rearrange("p (b t) -> p b t", t=T), axis=AXIS_X
                )
            nc.vector.scalar_tensor_tensor(
                out=stage_kj[:, k0 : k0 + G, 0], in0=ents, scalar=-BETA, in1=acc,
                op0=ALU.mult, op1=ALU.add,
            )

        # single store at the end on the sync ring (drained of loads by then)
        nc.sync.dma_start(out=out_view, in_=stage)

    nc.compile()
    return nc


_NC = None


def _get_nc():
    global _NC
    if _NC is None:
        _NC = build_nc(BC)
    return _NC


def _make_in_maps(inputs: dict) -> list[dict]:
    v = np.ascontiguousarray(np.asarray(inputs["values"], dtype=np.float32))
    lv = np.ascontiguousarray(np.asarray(inputs["last_value"], dtype=np.float32))
    r = np.ascontiguousarray(np.asarray(inputs["rewards"], dtype=np.float32))
    lp = np.ascontiguousarray(np.asarray(inputs["log_probs"], dtype=np.float32))
    en = np.ascontiguousarray(np.asarray(inputs["entropies"], dtype=np.float32))
    tm = np.ascontiguousarray(np.asarray(inputs["terminal_mask"]).astype(np.uint8))
    maps = []
    for c in range(N_CORES):
        sl = slice(c * BC, (c + 1) * BC)
        maps.append(
            {
                "values": v[sl],
                "last_value": lv[sl],
                "rewards": r[sl],
                "log_probs": lp[sl],
                "entropies": en[sl],
                "terminal_mask": tm[sl],
            }
        )
    return maps


def _run(inputs: dict, trace: bool = False):
    nc = _get_nc()
    res = run_bass_kernel_spmd(
        nc,
        _make_in_maps(inputs),
        core_ids=list(range(N_CORES)),
        trace=trace,
    )
    out = np.concatenate([res.results[c]["out"] for c in range(N_CORES)], axis=0)
    return out, res


def kernel(**inputs) -> np.ndarray:
    out, _ = _run(inputs, trace=False)
    return out
